# revision 1
# baseline (speedup 1.0000x reference)
"""AdaptiveGridMerger Trainium2 kernel.

Math: the reference scatters x[b,c,:] into a flat 8x8 grid with bilinear
(4-corner) weights from positions[b,c,:], then matmuls grid_weights
GW [270,64]. The scatter matrix S_b [64,306] (column c = the bilinear
hat weights of channel c) is tiny and depends only on positions, so it
is built on the HOST. The tail output rows 256:270 are folded into it:
  st78[c, 0:64]  = S_b[:, c]
  st78[c, 64:78] = (S_b.T @ GW[256:270].T)[c]   (Wtail fold)
so mm1 (lhsT=st78) produces gv[0:64] = S@x AND gv[64:78] = out[256:270]
in one pass. mm2 (lhsT=GW[0:256].T) produces out[0:256] from gv[0:64].

Device work is ONLY: 8 contiguous 128-partition read DMAs, bf16
matmuls, PSUM->SBUF cast copies (split DVE/ACT), 10 write DMAs. st and
gw ride inside the x2 read (extra columns) so there are no small
latency-bound DMAs to head-of-line block the x stream. All DMAs are on
the sync ring, reads strictly first (FIFO = read priority). The c1
chunk (last to arrive) is loaded in two column halves so the final
read unblocks only 4 matmuls + copies + one mm2 half.

Sharding: data-parallel over batch, 2 batches per core.

PSUM: one pool of 4 x [128,1024] f32 slots = exactly 8 banks; mm1
holds 4 gv accumulators per batch, mm2's o_ps tiles rotate through the
freed slots. Spin matmuls pre-ramp the PE clock during the DMA lead-in.
"""

import numpy as np

import concourse.bass as bass
import concourse.bacc as bacc
import concourse.mybir as mybir
from concourse import tile
from concourse.bass_utils import run_bass_kernel_spmd

B, C, T = 16, 306, 4096
M, G, GS = 270, 64, 8
N_CORES = 8
BL = B // N_CORES  # batches per core

W78 = G + 14          # st block width: 64 grid cols + 14 folded tail cols
XC = T // 2           # x2 packed region cols
SC = XC               # st blocks base col
GC = XC + 4 * W78     # gw half base col
WX2 = GC + 128        # x2st tensor width (2488)
T_PS = 512
N_SPIN = 7

MM_DTYPE = mybir.dt.bfloat16
NP_MM = mybir.dt.np(MM_DTYPE)
FP32 = mybir.dt.float32


def build_nc():
    nc = bacc.Bacc()
    x2st_ext = nc.declare_dram_parameter("x2st", [BL, 128, WX2], MM_DTYPE, isOutput=False)
    x01_ext = nc.declare_dram_parameter("x01", [BL, 2, 128, T], MM_DTYPE, isOutput=False)
    out_ext = nc.declare_dram_parameter("out", [BL, M, T], MM_DTYPE, isOutput=True)

    with tile.TileContext(nc) as tc:
        with (
            tc.tile_pool(name="const", bufs=1) as constp,
            tc.tile_pool(name="xp", bufs=1) as xp,
            tc.tile_pool(name="gvt", bufs=2) as gvtp,
            tc.tile_pool(name="op", bufs=2) as outp,
            tc.tile_pool(name="ps", bufs=4, space=bass.MemorySpace.PSUM) as psp,
        ):
            # PE clock pre-ramp while the first reads stream in.
            dummy = constp.tile([128, T_PS], MM_DTYPE, tag="dummy")
            nc.vector.memset(dummy[:], 0.0)
            spin_ps = psp.tile([128, 2 * T_PS], FP32, tag="pb", name="spin_ps")
            for _ in range(N_SPIN):
                nc.tensor.matmul(
                    spin_ps[:, :T_PS], dummy[:, :128], dummy[:], start=True, stop=True
                )

            # ---- reads, in the order compute consumes them
            x2st = {}
            xc0 = {}
            xc1 = {}
            for b in range(BL):
                x2st[b] = xp.tile([128, WX2], MM_DTYPE, tag=f"x2st{b}", name=f"x2st{b}")
                nc.sync.dma_start(out=x2st[b][:], in_=x2st_ext[b])
                xc0[b] = xp.tile([128, T], MM_DTYPE, tag=f"xc0_{b}", name=f"xc0_{b}")
                nc.sync.dma_start(out=xc0[b][:], in_=x01_ext[b, 0])
                xc1[b] = xp.tile([128, T], MM_DTYPE, tag=f"xc1_{b}", name=f"xc1_{b}")
                for tt in range(2):
                    nc.sync.dma_start(
                        out=xc1[b][:, tt * XC : (tt + 1) * XC],
                        in_=x01_ext[b, 1, :, tt * XC : (tt + 1) * XC],
                    )

            k_state = {"k": 0}

            def evac(dst, src):
                if k_state["k"] % 2 == 0:
                    nc.vector.tensor_copy(dst, src)
                else:
                    nc.scalar.copy(dst, src)
                k_state["k"] += 1

            out_sb = {}
            for b in range(BL):
                for mi in range(2):
                    out_sb[(b, mi)] = outp.tile(
                        [128, T], MM_DTYPE, tag=f"o{mi}", name=f"o{b}_{mi}"
                    )

            gvts = {}
            for b in range(BL):
                gvts[b] = gvtp.tile([W78, T], MM_DTYPE, tag="gvt", name=f"gvt{b}")

            gv = {}  # (b, w) -> live psum accumulator; w = 1024-col wave

            def quarter(b, w, q):
                return gv[(b, w)][:W78, q * T_PS : (q + 1) * T_PS]

            def mm1_chunk(b, waves, which, start, stop):
                for w in waves:
                    if (b, w) not in gv:
                        gv[(b, w)] = psp.tile(
                            [128, 2 * T_PS], FP32, tag="pb", name=f"gv{b}_{w}"
                        )
                for w in waves:
                    if which == 2:
                        p0 = 0 if w < 2 else 64
                        blk = 2 if w < 2 else 3
                        lhs = x2st[b][p0 : p0 + 50, SC + blk * W78 : SC + (blk + 1) * W78]
                        for q in range(2):
                            f0 = (w % 2) * 2 * T_PS + q * T_PS
                            nc.tensor.matmul(
                                quarter(b, w, q),
                                lhs,
                                x2st[b][p0 : p0 + 50, f0 : f0 + T_PS],
                                start=start, stop=stop, skip_group_check=True,
                            )
                    else:
                        lhs = x2st[b][0:128, SC + which * W78 : SC + (which + 1) * W78]
                        src = xc0[b] if which == 0 else xc1[b]
                        for q in range(2):
                            f0 = w * 2 * T_PS + q * T_PS
                            nc.tensor.matmul(
                                quarter(b, w, q),
                                lhs,
                                src[:, f0 : f0 + T_PS],
                                start=start, stop=stop, skip_group_check=True,
                            )

            def warm(b, waves, n):
                # zero-weight accumulates keep the PE activity monitor fed
                for s in range(n):
                    w = waves[(s // 2) % len(waves)]
                    nc.tensor.matmul(
                        quarter(b, w, s % 2),
                        dummy[:, :W78],
                        dummy[:],
                        start=False, stop=False, skip_group_check=True,
                    )

            def evac_waves(b, waves):
                for w in waves:
                    evac(
                        gvts[b][:W78, w * 2 * T_PS : (w + 1) * 2 * T_PS],
                        gv[(b, w)][:W78],
                    )
                    del gv[(b, w)]

            def mm2_half(b, tt):
                for mi in range(2):
                    for h in range(2):
                        o_ps = psp.tile([128, 2 * T_PS], FP32, tag="pb", name="o_ps")
                        c0 = tt * 4 * T_PS + h * 2 * T_PS
                        for q in range(2):
                            nc.tensor.matmul(
                                o_ps[:, q * T_PS : (q + 1) * T_PS],
                                x2st[mi][0:G, GC : GC + 128],
                                gvts[b][:G, c0 + q * T_PS : c0 + (q + 1) * T_PS],
                                start=True, stop=True, skip_group_check=True,
                            )
                        # zero-add filler: keeps the PE activity monitor fed
                        # while the next PSUM slot waits on a copy
                        nc.tensor.matmul(
                            o_ps[:, 0:T_PS],
                            dummy[:, :128],
                            dummy[:],
                            start=False, stop=False, skip_group_check=True,
                        )
                        evac(out_sb[(b, mi)][:, c0 : c0 + 2 * T_PS], o_ps[:])
                for mi in range(2):
                    nc.sync.dma_start(
                        out=out_ext[b, mi * 128 : (mi + 1) * 128, tt * XC : (tt + 1) * XC],
                        in_=out_sb[(b, mi)][:, tt * XC : (tt + 1) * XC],
                    )

            # ---- batch 0: full mm1, evac per T-half
            mm1_chunk(0, (0, 1, 2, 3), 2, True, False)
            warm(0, (0, 1, 2, 3), 8)
            mm1_chunk(0, (0, 1, 2, 3), 0, False, False)
            mm1_chunk(0, (0, 1), 1, False, True)
            evac_waves(0, (0, 1))
            mm1_chunk(0, (2, 3), 1, False, True)
            evac_waves(0, (2, 3))
            nc.sync.dma_start(out=out_ext[0, 256:M, :], in_=gvts[0][G:W78, :])

            # ---- mm2(b0); then b1's mm1 in T-half phases (2 PSUM slots
            # ---- each, so they rotate in without waiting all b0 copies)
            mm2_half(0, 0)
            mm2_half(0, 1)
            for tt in range(2):
                ws = (0, 1) if tt == 0 else (2, 3)
                mm1_chunk(1, ws, 2, True, False)
                warm(1, ws, 2)
                mm1_chunk(1, ws, 0, False, False)
                mm1_chunk(1, ws, 1, False, True)
                evac_waves(1, ws)
            nc.sync.dma_start(out=out_ext[1, 256:M, :], in_=gvts[1][G:W78, :])
            mm2_half(1, 0)
            mm2_half(1, 1)
    nc.compile()
    return nc


def _host_st(positions, grid_weights):
    """st78 [B, C, 78] f32: bilinear hat weights + folded tail rows."""
    gp = (positions.astype(np.float32) + 1.0) * (GS / 2.0)  # [B, C, 2]
    i = np.arange(GS, dtype=np.float32)
    wy = np.maximum(0.0, 1.0 - np.abs(i[None, None, :] - gp[:, :, 0:1]))
    wx = np.maximum(0.0, 1.0 - np.abs(i[None, None, :] - gp[:, :, 1:2]))
    s = (wy[:, :, :, None] * wx[:, :, None, :]).reshape(B, C, G)
    wtail = s @ grid_weights[256:M].T.astype(np.float32)  # [B, C, 14]
    return np.concatenate([s, wtail], axis=2)


def make_in_maps(x, positions, grid_weights):
    st78 = _host_st(positions, grid_weights)
    gw = np.ascontiguousarray(grid_weights[:256].T).astype(NP_MM)  # [64, 256]
    x_mm = x.astype(NP_MM)
    in_maps = []
    for i in range(N_CORES):
        sl = slice(i * BL, (i + 1) * BL)
        x2st_pack = np.zeros((BL, 128, WX2), dtype=np.float32)
        for b2 in range(BL):
            gb = i * BL + b2
            xc2 = x_mm[gb, 256:C].astype(np.float32).reshape(50, 2, XC)
            x2st_pack[b2, 0:50, 0:XC] = xc2[:, 0]
            x2st_pack[b2, 64:114, 0:XC] = xc2[:, 1]
            x2st_pack[b2, :, SC : SC + W78] = st78[gb, 0:128]
            x2st_pack[b2, :, SC + W78 : SC + 2 * W78] = st78[gb, 128:256]
            x2st_pack[b2, 0:50, SC + 2 * W78 : SC + 3 * W78] = st78[gb, 256:C]
            x2st_pack[b2, 64:114, SC + 3 * W78 : SC + 4 * W78] = st78[gb, 256:C]
            x2st_pack[b2, 0:64, GC : GC + 128] = gw[:, b2 * 128 : (b2 + 1) * 128]
        in_maps.append(
            {
                "x2st": x2st_pack.astype(NP_MM),
                "x01": np.ascontiguousarray(x_mm[sl, 0:256]).reshape(BL, 2, 128, T),
            }
        )
    return in_maps


_NC_CACHE = None


def kernel(x, positions, grid_weights):
    global _NC_CACHE
    if _NC_CACHE is None:
        _NC_CACHE = build_nc()
    nc = _NC_CACHE
    in_maps = make_in_maps(x, positions, grid_weights)
    res = run_bass_kernel_spmd(nc, in_maps, core_ids=list(range(N_CORES)))
    out = np.concatenate([r["out"] for r in res.results], axis=0)
    return np.asarray(out, dtype=np.float32)


if __name__ == "__main__":
    xs = np.random.randn(B, C, T).astype(np.float32)
    ps = np.random.uniform(-1, 0.74, (B, C, 2)).astype(np.float32)
    gw = np.random.randn(M, G).astype(np.float32)
    out = kernel(xs, ps, gw)
    print(out.shape, out.dtype)

